# revision 49
# baseline (speedup 1.0000x reference)
"""Low-rank self-attention on 8 trn2 NeuronCores.

reference math (per batch b):
  q = x @ Wq.T            [S,R]
  k = x @ Wk.T            [S,R]
  v = x @ Wv.T            [S,D]
  P = softmax(q k^T / sqrt(R))    (mask is all-ones -> no-op)
  out = (P v) @ Wo.T      [S,D]

Key algebraic fold: (P (x Wv^T)) Wo^T = P (x (Wv^T Wo^T)) = P (x W2).
W2 = Wv^T Wo^T is precomputed on the host once per weight set, removing
the entire output projection from the device (out = P v2, v2 = x W2).

Sharding: 8 cores = (batch b in 0..3) x (e-half h in 0..1).  Each core
computes the FULL attention matrix for its batch but only its 512-wide
half of the output features: v2h = x @ W2[:, h-half], out_h = P v2h.
Splitting e (not queries) halves the dominant x@W2 projection per core;
the duplicated q/k-proj + scores are much cheaper.

fp8 DoubleRow projections: all three x-projections (q, k, v2) run as
fp8e4m3 DoubleRow matmuls (2 contraction rows/partition, 0.5 PE
cycles/row = 4x bf16 throughput) with first-order error compensation:
  x  = x0 + x1/32           (x0 = fp8(x), x1 = fp8(32(x-x0)))
  W  = W0 + W1/32           (host-exact split)
  32(xW) ~= x0(32 W0) + x1 W0 + x0 W1     [drops x1W1/1024 ~ 0.1%]
All 12 DoubleRow matmuls accumulate into one PSUM group (12*256=3072
PE-cycles vs 4096 bf16).  The x32 output scaling is folded away for
free: q/k keep it (scores become 1024x, absorbed into the exp scale)
and for v2 the rowsum 'ones' vector is 32.0 so 1/s cancels it.

On chip:
  qT [128r, 2048q]=32q , kT=32k , v2h[kt] [128k, 512e]=32*v2h  (bf16)
  scoresT[k,q] = kT_chunk.T @ qT  -> exp(x/1024) (no max-subtract)
  s[q] = sum_k 32*E[k,q] via tiny matmuls E.T @ (32*ones)
  ctx[q,e] = sum_kt E[kt].T-block @ v2h[kt]  = 32*ctx  (accum PSUM)
  out[q,e] = ctx * (1/s[q]) per partition -> DMA   (32s cancel)
"""

import math
import sys

import numpy as np

for _p in ("/opt/trn_rl_repo",):
    if _p not in sys.path:
        sys.path.append(_p)

import ml_dtypes  # noqa: E402

B, S, D, R = 4, 2048, 1024, 128
EH = D // 2          # output-feature columns per core
NCORES = 8
NDT = D // 128       # 8 d-tiles
NPT = NDT // 2       # 4 DoubleRow pair-tiles
NKT = S // 128       # 16 k-tiles
NQC = S // 512       # 4 q-chunks (full batch per core)
SCALE = 1.0 / math.sqrt(R)
FP8_NP = ml_dtypes.float8_e4m3fn

_CACHE = {}


def _build(dt_np):
    import concourse.bass as bass  # noqa: F401
    import concourse.tile as tile
    from concourse import bacc, mybir

    DT = mybir.dt.from_np(np.dtype(dt_np))
    FP8 = mybir.dt.float8e4
    F32 = mybir.dt.float32
    Exp = mybir.ActivationFunctionType.Exp
    DR = mybir.MatmulPerfMode.DoubleRow

    nc = bacc.Bacc(
        "TRN2", target_bir_lowering=False, debug=False,
        enable_asserts=False, num_devices=NCORES,
    )
    # fp8 DoubleRow pair layout: [128p, (variant), pair-tile, 2, n]
    # x is chunk-major so each 512-col chunk is one contiguous DMA
    xv_d = nc.dram_tensor("xv", [128, 4, 2, NPT, 2, 512], FP8,
                          kind="ExternalInput").ap()
    wqv_d = nc.dram_tensor("wqv", [128, 3, NPT, 2, R], FP8, kind="ExternalInput").ap()
    wkv_d = nc.dram_tensor("wkv", [128, 3, NPT, 2, R], FP8, kind="ExternalInput").ap()
    w2v_ds = [nc.dram_tensor(f"w2v{v}", [128, NPT, 2, EH], FP8,
                             kind="ExternalInput").ap() for v in range(3)]
    out_d = nc.dram_tensor("out", [S, EH], F32, kind="ExternalOutput").ap()

    from contextlib import ExitStack

    with tile.TileContext(nc) as tc, ExitStack() as es:
        pw = es.enter_context(tc.tile_pool(name="pw", bufs=1))
        px = es.enter_context(tc.tile_pool(name="px", bufs=1))
        pv = es.enter_context(tc.tile_pool(name="pv", bufs=1))
        pqk = es.enter_context(tc.tile_pool(name="pqk", bufs=1))
        pE = es.enter_context(tc.tile_pool(name="pE", bufs=1))
        posb = es.enter_context(tc.tile_pool(name="posb", bufs=3))
        prs = es.enter_context(tc.tile_pool(name="prs", bufs=4))
        ps_sc = es.enter_context(tc.tile_pool(name="ps_sc", bufs=2, space="PSUM"))
        ps_sc2 = es.enter_context(tc.tile_pool(name="ps_sc2", bufs=1, space="PSUM"))
        ps_v = es.enter_context(tc.tile_pool(name="ps_v", bufs=2, space="PSUM"))
        ps_ctx = es.enter_context(tc.tile_pool(name="ps_ctx", bufs=2, space="PSUM"))

        mm = nc.tensor.matmul
        cp = nc.vector.tensor_copy

        # ---- persistent inputs -------------------------------------------
        wqv = pw.tile([128, 3, NPT, 2, R], FP8, name="wqv")
        wkv = pw.tile([128, 3, NPT, 2, R], FP8, name="wkv")
        w2vs = [pw.tile([128, NPT, 2, EH], FP8, name=f"w2v{v}") for v in range(3)]
        xv = px.tile([128, 4, 2, NPT, 2, 512], FP8, name="xv")

        def xchunk(c, split=1):
            # chunk-major layout: each chunk is contiguous in both spaces
            step = 2 // split
            for s0 in range(0, 2, step):
                nc.sync.dma_start(out=xv[:, c, s0:s0 + step],
                                  in_=xv_d[:, c, s0:s0 + step])

        # DMA order = consumption order: x0/x1 of chunk 0 split around wkv
        # (qproj's x0-mms run while wkv/x1 stream); w2v split per variant in
        # vproj's consumption order (32W0, W0, W1) behind chunk 1.
        nc.sync.dma_start(out=wqv, in_=wqv_d)
        nc.sync.dma_start(out=xv[:, 0, 0:1, 0:2], in_=xv_d[:, 0, 0:1, 0:2])
        nc.sync.dma_start(out=xv[:, 0, 0:1, 2:4], in_=xv_d[:, 0, 0:1, 2:4])
        nc.sync.dma_start(out=wkv, in_=wkv_d)
        nc.sync.dma_start(out=xv[:, 0, 1:2], in_=xv_d[:, 0, 1:2])
        xchunk(1)
        for v in (0, 2, 1):  # vproj consumption order: 32*W0, W0, W1
            nc.sync.dma_start(out=w2vs[v], in_=w2v_ds[v])
        xchunk(2)
        xchunk(3)
        # warm-up tile: content is irrelevant (results never consumed); a
        # one-column memset satisfies the written-before-read requirement
        # while keeping the warm matmuls' dependency latency tiny
        warm = pw.tile([128, 512], DT, name="warm")
        nc.vector.memset(warm[:, 0:1], 0.0)
        ones = pw.tile([128, 512], DT, name="ones")
        nc.vector.memset(ones, 32.0)

        qT = pqk.tile([128, S], DT, name="qT")
        kT = pqk.tile([128, S], DT, name="kT")
        vt = [pv.tile([128, EH], DT, name=f"v{k}") for k in range(NKT)]
        # E stored as kt-pairs [128k, 1024] (two 512q halves) so one wide
        # activation serves two score tiles; Eq(qc, kt) slices the q-subtile
        E2s = [[None] * (NKT // 2) for _ in range(NQC)]

        def Eq(qc, kt, j0, j1):
            return E2s[qc][kt // 2][:, (kt % 2) * 512 + j0:(kt % 2) * 512 + j1]

        # ---- fp8 DoubleRow compensated projections -----------------------
        # psum += lhs0*(32 w0) + lhs1*w0 + lhs0*w1  (= 32 * x@W exactly to
        # first order; variants v: 0 = 32*W0, 1 = W1, 2 = W0)
        def qkproj(wv, dst, c, nm):
            ps = ps_sc.tile([128, 512], F32, name=f"qk{nm}_{c}", tag="scps")
            x0 = xv[:, c, 0]
            x1 = xv[:, c, 1]
            n = 0
            for wsel, xval in ((0, x0), (1, x0), (2, x1)):
                for i in range(NPT):
                    mm(ps, lhsT=wv[:, wsel, i], rhs=xval[:, i],
                       start=(n == 0), stop=(n == 3 * NPT - 1), perf_mode=DR)
                    n += 1
            cp(dst[:, c * 512:(c + 1) * 512], ps)

        def vproj(kt):
            ps = ps_v.tile([128, 512], F32, name=f"v_ps{kt}", tag="vps")
            c, o = divmod(kt, 4)
            x0 = xv[:, c, 0, :, :, o * 128:(o + 1) * 128]
            x1 = xv[:, c, 1, :, :, o * 128:(o + 1) * 128]
            n = 0
            for wsel, xval in ((0, x0), (2, x1), (1, x0)):
                for i in range(NPT):
                    mm(ps, lhsT=xval[:, i], rhs=w2vs[wsel][:, i],
                       start=(n == 0), stop=(n == 3 * NPT - 1), perf_mode=DR)
                    n += 1
            cp(vt[kt], ps)

        def score2(qc, kt2):
            # two k-tiles' scores into one 2-bank psum tile, one wide exp
            sc = ps_sc2.tile([128, 1024], F32, name=f"sc{qc}_{kt2}", tag="scps2")
            for h in range(2):
                mm(sc[:, h * 512:(h + 1) * 512],
                   lhsT=kT[:, (2 * kt2 + h) * 128:(2 * kt2 + h + 1) * 128],
                   rhs=qT[:, qc * 512:(qc + 1) * 512], start=True, stop=True)
            Ek = pE.tile([128, 1024], DT, name=f"E{qc}_{kt2}")
            # qT/kT hold 32q/32k -> scores are 1024x; absorb into exp scale
            nc.scalar.activation(Ek, sc, Exp, scale=SCALE / 1024.0)
            E2s[qc][kt2] = Ek

        # Warm-up: PE matmuls while the first DMAs land.  Keeps the PE
        # continuously busy from ~0.1us so the p-state ramp (full clock only
        # after 3us of busy) completes before real work.
        for w in range(9):
            wps = ps_ctx.tile([1, 512], F32, name=f"warm{w}", tag="ctxps")
            mm(wps, lhsT=warm[:, 0:1], rhs=warm, start=True, stop=True)

        # PE emission order tracks xt chunk-arrival; per chunk c we can run
        # qproj(c), kproj(c), and all newly-unlocked scores.  vproj is the
        # deferrable PE filler: emit just enough per chunk to cover the DMA
        # cadence, back-loading the rest so the final chunk's 28-score exp
        # burst (Act-bound) overlaps trailing vprojs instead of stalling PE.
        vq = iter(range(NKT))
        vbudget = [0, 6, 4, 6]
        for c in range(4):
            qkproj(wqv, qT, c, "q")
            qkproj(wkv, kT, c, "k")
            # newly unlocked score-pairs: (qc < c, pairs of chunk c) and
            # (qc == c, all pairs <= chunk c); interleave vproj filler
            vleft = vbudget[c]
            todo = [(qc, kt2) for qc in range(c) for kt2 in (2 * c, 2 * c + 1)]
            todo += [(c, kt2) for kc in range(c + 1) for kt2 in (2 * kc, 2 * kc + 1)]
            for n, (qc, kt2) in enumerate(todo):
                score2(qc, kt2)
                if n % 2 == 1 and vleft > 0:
                    vproj(next(vq))
                    vleft -= 1
            for _ in range(vleft):
                vproj(next(vq))

        # ---- rowsums + attention context ---------------------------------
        # s' = 32*s via ones=32; ctx' = 32*ctx via v2h scale; 1/s' * ctx' = out
        rss = []
        for qc in range(NQC):
            # one accumulation group for the whole bank: start=True clears
            # has_written for the entire bank, so only the very first mm may
            # set it; later cols overwrite-then-accumulate.
            s_ps = ps_sc.tile([128, 4], F32, name=f"s_ps{qc}", tag="scps")
            for kt in range(NKT):
                for j in range(4):
                    mm(s_ps[:, j:j + 1],
                       lhsT=Eq(qc, kt, j * 128, (j + 1) * 128),
                       rhs=ones[:, 0:1], start=(kt == 0 and j == 0),
                       stop=(kt == NKT - 1 and j == 3))
            rs = prs.tile([128, 4], F32, name=f"rs{qc}", tag="rs")
            nc.vector.reciprocal(rs, s_ps)
            rss.append(rs)

        for qc in range(NQC):
            for qs in range(4):
                last = (qc == NQC - 1 and qs == 3)
                q0 = qc * 512 + qs * 128
                if not last:
                    ops = ps_ctx.tile([128, EH], F32, name=f"c{qc}_{qs}",
                                      tag="ctxps")
                    for kt in range(NKT):
                        mm(ops, lhsT=Eq(qc, kt, qs * 128, (qs + 1) * 128),
                           rhs=vt[kt], start=(kt == 0), stop=(kt == NKT - 1))
                    osb = posb.tile([128, EH], F32, name=f"osb{qc}_{qs}",
                                    tag="osb")
                    nc.scalar.mul(osb, ops, rss[qc][:, qs:qs + 1])
                    nc.sync.dma_start(out=out_d[q0:q0 + 128, :], in_=osb)
                else:
                    # split the final pass so the trailing mul+DMA are small
                    for eh in range(2):
                        ops = ps_ctx.tile([128, EH // 2], F32,
                                          name=f"c{qc}_{qs}_{eh}", tag="ctxps")
                        esl = slice(eh * (EH // 2), (eh + 1) * (EH // 2))
                        for kt in range(NKT):
                            mm(ops, lhsT=Eq(qc, kt, qs * 128, (qs + 1) * 128),
                               rhs=vt[kt][:, esl], start=(kt == 0),
                               stop=(kt == NKT - 1))
                        osb = posb.tile([128, EH // 2], F32,
                                        name=f"osb{qc}_{qs}_{eh}", tag="osbh")
                        nc.scalar.mul(osb, ops, rss[qc][:, qs:qs + 1])
                        nc.sync.dma_start(out=out_d[q0:q0 + 128, esl], in_=osb)

    nc.compile()
    return nc


def _fp8_split(arr32):
    """arr -> (fp8(arr), fp8(32*(arr - fp8(arr)))) pair."""
    a0 = arr32.astype(FP8_NP)
    a1 = (32.0 * (arr32 - a0.astype(np.float32))).astype(FP8_NP)
    return a0, a1


def _pair_tiles(mat, n):
    """[D, n] -> [128, NPT, 2, n] DoubleRow pair layout, f32."""
    return np.ascontiguousarray(
        mat.reshape(NPT, 2, 128, n).transpose(2, 0, 1, 3).astype(np.float32))


def _weight_variants(mat, n):
    """[D, n] -> [128, 3, NPT, 2, n] fp8: (32*W0, W1, W0)."""
    p = _pair_tiles(mat, n)
    w0, w1 = _fp8_split(p)
    w0s = (32.0 * w0.astype(np.float32)).astype(FP8_NP)
    return np.ascontiguousarray(np.stack([w0s, w1, w0], axis=1))


def _prep_inputs(x, Wq, Wk, Wv, Wo, dt_np):
    """Host-side shard + fp8 split + transpose. Returns per-core inputs."""
    wqv = _weight_variants(Wq.T.astype(np.float32), R)
    wkv = _weight_variants(Wk.T.astype(np.float32), R)
    # fold the output projection into the value projection: out = P (x W2)
    W2 = (Wo.astype(np.float32) @ Wv.astype(np.float32)).T
    w2vh = []
    for h in range(2):
        p = _pair_tiles(np.ascontiguousarray(W2[:, h * EH:(h + 1) * EH]), EH)
        w0, w1 = _fp8_split(p)
        w0s = (32.0 * w0.astype(np.float32)).astype(FP8_NP)
        w2vh.append({"w2v0": w0s, "w2v1": np.ascontiguousarray(w1),
                     "w2v2": np.ascontiguousarray(w0)})
    xvs = []
    for b in range(B):
        p = _pair_tiles(np.ascontiguousarray(x[b].T), S)
        x0, x1 = _fp8_split(p)
        v = np.stack([x0, x1], axis=1)          # [128, 2, NPT, 2, S]
        v = v.reshape(128, 2, NPT, 2, 4, 512)   # chunk-major for DMA
        xvs.append(np.ascontiguousarray(v.transpose(0, 4, 1, 2, 3, 5)))
    in_maps = []
    for c in range(NCORES):
        b, h = divmod(c, 2)
        in_maps.append({"xv": xvs[b], "wqv": wqv, "wkv": wkv, **w2vh[h]})
    return in_maps


def _run(inputs, dt_np=ml_dtypes.bfloat16, trace=False, **kw):
    from concourse.bass_utils import run_bass_kernel_spmd

    key = np.dtype(dt_np).str
    if key not in _CACHE:
        _CACHE[key] = _build(dt_np)
    nc = _CACHE[key]
    in_maps = _prep_inputs(inputs["x"], inputs["Wq"], inputs["Wk"],
                           inputs["Wv"], inputs["Wo"], dt_np)
    res = run_bass_kernel_spmd(nc, in_maps, core_ids=list(range(NCORES)),
                               trace=trace, **kw)
    out = np.empty((B, S, D), np.float32)
    for c in range(NCORES):
        b, h = divmod(c, 2)
        out[b, :, h * EH:(h + 1) * EH] = res.results[c]["out"]
    return out, res


def kernel(x, mask, Wq, Wk, Wv, Wo):
    # mask is all-ones by construction (spec fill=ones) -> identity.
    out, _ = _run({"x": np.asarray(x, np.float32), "Wq": np.asarray(Wq, np.float32),
                   "Wk": np.asarray(Wk, np.float32), "Wv": np.asarray(Wv, np.float32),
                   "Wo": np.asarray(Wo, np.float32)})
    return out
